# revision 21
# baseline (speedup 1.0000x reference)
"""Trainium2 Bass kernel for nn_Cross_AttentionHead_withMask.

Cross-attention head: q = rope(x_text @ Wq.T), k = rope2d(x_image @ Wk.T),
v = x_image @ Wv.T, out = softmax(q k^T / sqrt(512)) v.
(x_latex_mask is accepted but unused — it is dead in the reference.)

Sharding: data-parallel over batch B=8, one batch per NeuronCore (8 cores).

Per-core device program (all matmuls bf16, accumulation/softmax stats fp32).
Schedule is Act-engine-bound: the exp over the [2048, 4096] score matrix is
the serial floor (~64 x 1.1us), so everything else is arranged to hide under
it:
  - K2 packed as [128, TK/2]: rows 0:64 = K.T for t-tiles 0..15, rows 64:128
    = t-tiles 16..31 (k-proj writes PSUM at partition offset 64 for the hi
    half). A scores step pairs t-tiles (P, P+16) via the two PE row groups.
  - steps run (sc0,P),(sc1,P) for P=0..15, then (sc2,P),(sc3,P): per-chunk
    k/v-proj + rope work is spread over 8 steps, under the Act cadence.
  - scores for step i+1 are emitted before exp(i) so the Act engine never
    waits on the PE.
  - head dim is permuted to evens-then-odds so RoPE pairs become row blocks;
    rope = A*CC + partner(A)*SS where partner = cross-partition copies.
  - attention-out: psoT[h, s] += v_aug[t-tile].T @ expT, with a ones column
    in v_aug accumulating the softmax denominator for free.
  - epilogue per s-chunk: PE-transpose [65, 128] -> [128, 65], reciprocal of
    the Z column, tensor_scalar multiply, DMA out.
"""
import numpy as np
from contextlib import ExitStack

import ml_dtypes

B, TQ, TK = 8, 2048, 4096
DIM_IMG, DIM_TXT, HS = 512, 128, 64
N_CORES = 8
SCALE = float(DIM_IMG) ** -0.5  # reference scales by sqrt(image embed dim)

BF16 = ml_dtypes.bfloat16

_prog_cache = {}


def _patch_tile_drain():
    """This walrus build rejects a Drain carrying >1 sem wait; split the
    TileContext exit waits onto one-wait NoOps."""
    import concourse.tile as tile
    from concourse import mybir
    from concourse.vector_clock import ScopedClock

    if getattr(tile.TileContext, "_drain_patched", False):
        return

    def _drain_and_barrier(self, tick_clock, wait_clock):
        nc = self.nc
        nop = nc.sync.nop()
        wait_clock.add_sem_waits(nop.ins, ScopedClock({None: tick_clock.global_clock}))
        si = nop.ins.sync_info
        waits = list(si.on_wait) if si is not None else []
        if len(waits) > 1:
            nop.ins.sync_info = mybir.SyncInfo(on_wait=[waits[0]], on_update=[])
            for w in waits[1:]:
                extra = nc.sync.nop()
                extra.ins.sync_info = mybir.SyncInfo(on_wait=[w], on_update=[])
        nc.sync.drain()
        nc.all_engine_barrier()
        assert self.sems is not None
        popped = nc._tile_sem_poison_stack.pop()
        assert popped is self._sem_poison
        nc.clear_and_free_semaphores(list(self.sems.allocated().values()))
        nc.all_engine_barrier()

    tile.TileContext._drain_and_barrier = _drain_and_barrier
    tile.TileContext._drain_patched = True


def _split_excess_waits(nc):
    """This walrus build caps sem waits per instruction (1 for DMA/Drain-style
    control instructions, 2 for compute). Move excess waits onto same-engine
    NoOps inserted right before the offending instruction — the engine queue
    is FIFO, so blocking dispatch on the NoOp is semantically equivalent."""
    from concourse import mybir

    ctr = 0
    for fn in nc.m.functions:
        for b in fn.blocks:
            il = b.instructions
            new = []
            changed = False
            for inst in il:
                si = inst.sync_info
                waits = list(si.on_wait) if si is not None else []
                lim = 1
                if len(waits) > lim:
                    for w in waits[lim:]:
                        nop = mybir.InstNoOp(name=f"wsplit-{ctr}", ins=[], outs=[])
                        ctr += 1
                        nop.engine = inst.engine
                        nop.sync_info = mybir.SyncInfo(on_wait=[w], on_update=[])
                        new.append(nop)
                    inst.sync_info = mybir.SyncInfo(
                        on_wait=waits[:lim], on_update=list(si.on_update)
                    )
                    changed = True
                new.append(inst)
            if changed:
                b.instructions = new


def build_program(split_waits=True):
    """Build the single-core Bass program (same program runs SPMD on 8 cores)."""
    key = ("nc", split_waits)
    if key in _prog_cache:
        return _prog_cache[key]

    _patch_tile_drain()
    import concourse.bass as bass
    import concourse.tile as tile
    from concourse import mybir
    from concourse.masks import make_identity

    FP = mybir.dt.float32
    BF = mybir.dt.bfloat16

    nc = bass.Bass("TRN2", target_bir_lowering=False, debug=False)
    xt = nc.dram_tensor("xt", [DIM_IMG, TK], BF, kind="ExternalInput").ap()
    xtt = nc.dram_tensor("xtt", [DIM_TXT, TQ], BF, kind="ExternalInput").ap()
    wk = nc.dram_tensor("wk", [DIM_IMG, HS], BF, kind="ExternalInput").ap()
    wq = nc.dram_tensor("wq", [DIM_TXT, HS], BF, kind="ExternalInput").ap()
    wv = nc.dram_tensor("wv", [DIM_IMG, HS], BF, kind="ExternalInput").ap()
    # folded rope tables: [128, TK/2]; rows 0:64 = t 0..2047, 64:128 = rest
    cck = nc.dram_tensor("cck", [128, TK // 2], BF, kind="ExternalInput").ap()
    ssk = nc.dram_tensor("ssk", [128, TK // 2], BF, kind="ExternalInput").ap()
    ccq = nc.dram_tensor("ccq", [HS, TQ], BF, kind="ExternalInput").ap()
    ssq = nc.dram_tensor("ssq", [HS, TQ], BF, kind="ExternalInput").ap()
    out = nc.dram_tensor("out", [TQ, HS], FP, kind="ExternalOutput").ap()

    Exp = mybir.ActivationFunctionType.Exp
    NC4 = DIM_IMG // 128  # 4 c-chunks
    NT = TK // 128  # 32 t-tiles
    NP = NT // 2  # 16 scores pairs (tile P with tile P+16)
    NCP = 4  # chunk-pairs: cp p = t-cols [512p:512p+512] and [2048+512p : ...]

    with tile.TileContext(nc) as tc:
        with ExitStack() as ctx:
            const = ctx.enter_context(tc.tile_pool(name="const", bufs=1))
            pwp = ctx.enter_context(tc.tile_pool(name="pw", bufs=2, space="PSUM"))
            projp = ctx.enter_context(tc.tile_pool(name="proj", bufs=2, space="PSUM"))
            pop = ctx.enter_context(tc.tile_pool(name="po", bufs=2, space="PSUM"))
            esb = ctx.enter_context(tc.tile_pool(name="esb", bufs=4))
            osbp = ctx.enter_context(tc.tile_pool(name="osb", bufs=2))
            ktp = ctx.enter_context(tc.tile_pool(name="ktp", bufs=2))
            pkp = ctx.enter_context(tc.tile_pool(name="pkp", bufs=2))

            # ---- DMA: each dma_start is a ~0.7us DIRECT2D on its host
            # sequencer, issued serially per engine — so use FEW, LARGE
            # descriptors and spread the startup-critical wave across five
            # engines' rings so they issue in parallel. ----
            wq_sb = const.tile([128, HS], BF, tag="wq")
            xtt_sb = const.tile([128, TQ], BF, tag="xtt")
            wk_sb = const.tile([128, NC4 * HS], BF, tag="wk")
            wv_sb = const.tile([128, NC4 * HS], BF, tag="wv")
            ccq_sb = const.tile([HS, TQ], BF, tag="ccq")
            ssq_sb = const.tile([HS, TQ], BF, tag="ssq")
            cck_sb = const.tile([128, TK // 2], BF, tag="cck")
            ssk_sb = const.tile([128, TK // 2], BF, tag="ssk")
            xt_sb = [const.tile([128, TK], BF, tag=f"xt{ci}", name=f"xt_sb{ci}")
                     for ci in range(NC4)]

            def xt_cp_dma(eng, p, ci):
                # both t-col halves of chunk-pair p for one c-chunk: [128,2,512]
                src = xt[ci * 128:(ci + 1) * 128, :].rearrange(
                    "part (g t) -> part g t", g=2)[:, :, p * 512:(p + 1) * 512]
                dst = xt_sb[ci][:, :].rearrange(
                    "part (g t) -> part g t", g=2)[:, :, p * 512:(p + 1) * 512]
                eng.dma_start(dst, src)

            # The DMA engines drain transfers roughly in post order at
            # ~360GB/s aggregate, so the startup-critical wave (~1.9MB:
            # q inputs + chunk-pair 0 + its tables) is posted first on all
            # three rings, then the bulk in consolidated descriptors.
            def xt_rest_dma(eng, ci):
                src = xt[ci * 128:(ci + 1) * 128, :].rearrange(
                    "part (g t) -> part g t", g=2)[:, :, 512:2048]
                dst = xt_sb[ci][:, :].rearrange(
                    "part (g t) -> part g t", g=2)[:, :, 512:2048]
                eng.dma_start(dst, src)

            # scalar ring: q-proj chain + q tables (Act consumes them first)
            nc.scalar.dma_start(wq_sb[:], wq[:])
            nc.scalar.dma_start(xtt_sb[:, 0:1024], xtt[:, 0:1024])
            nc.scalar.dma_start(ccq_sb[:, 0:1024], ccq[:, 0:1024])
            nc.scalar.dma_start(ssq_sb[:, 0:1024], ssq[:, 0:1024])
            # sync ring: critical wave then bulk
            nc.sync.dma_start(
                wk_sb[:].rearrange("p (a h) -> p a h", a=NC4),
                wk.rearrange("(a p) h -> p a h", p=128),
            )
            xt_cp_dma(nc.sync, 0, 0)
            xt_cp_dma(nc.sync, 0, 1)
            xt_rest_dma(nc.sync, 0)
            xt_rest_dma(nc.sync, 1)
            nc.sync.dma_start(cck_sb[:, 512:2048], cck[:, 512:2048])
            nc.sync.dma_start(ssk_sb[:, 512:2048], ssk[:, 512:2048])
            nc.sync.dma_start(xtt_sb[:, 1024:2048], xtt[:, 1024:2048])
            # gpsimd ring: critical wave then bulk
            nc.gpsimd.dma_start(cck_sb[:, 0:512], cck[:, 0:512])
            nc.gpsimd.dma_start(ssk_sb[:, 0:512], ssk[:, 0:512])
            xt_cp_dma(nc.gpsimd, 0, 2)
            xt_cp_dma(nc.gpsimd, 0, 3)
            nc.gpsimd.dma_start(
                wv_sb[:].rearrange("p (a h) -> p a h", a=NC4),
                wv.rearrange("(a p) h -> p a h", p=128),
            )
            xt_rest_dma(nc.gpsimd, 2)
            xt_rest_dma(nc.gpsimd, 3)
            # late q tables (needed from phase B, ~35us in)
            nc.gpsimd.dma_start(ccq_sb[:, 1024:2048], ccq[:, 1024:2048])
            nc.gpsimd.dma_start(ssq_sb[:, 1024:2048], ssq[:, 1024:2048])

            ident = const.tile([128, 128], FP, tag="ident")
            make_identity(nc, ident[:])
            identb = const.tile([128, 128], BF, tag="identb")
            nc.gpsimd.tensor_copy(identb[:], ident[:])

            K2f = const.tile([128, TK // 2], BF, tag="K2f")
            Q2 = const.tile([128, TQ], BF, tag="Q2")
            qt_pre = const.tile([HS, TQ], BF, tag="qtpre")
            v_half = [const.tile([128, (NT // 2) * 65], BF, tag=f"vall{h}",
                                 name=f"vall{h}") for h in range(2)]
            nc.gpsimd.memset(v_half[0][:, HS::65], 1.0)
            nc.gpsimd.memset(v_half[1][:, HS::65], 1.0)

            # ---- q projection (Act copies: Act is idle until the first exp)
            def q_proj_chunk(j, cp):
                ps = projp.tile([HS, 512], FP, tag="proj", name=f"psq{j}")
                nc.tensor.matmul(
                    ps[:], lhsT=wq_sb[:], rhs=xtt_sb[:, j * 512:(j + 1) * 512],
                    start=True, stop=True,
                )
                cp(qt_pre[:, j * 512:(j + 1) * 512], ps[:])

            def q_rope_chunk(j):
                cs = slice(j * 512, (j + 1) * 512)
                pkq = pkp.tile([HS, 512], BF, tag="pkq", name=f"pkq{j}")
                nc.vector.tensor_copy(pkq[0:32, :], qt_pre[32:64, cs])
                nc.vector.tensor_copy(pkq[32:64, :], qt_pre[0:32, cs])
                t1 = pkp.tile([HS, 512], BF, tag="t1q", name=f"t1q{j}")
                nc.vector.tensor_mul(t1[:], qt_pre[:, cs], ccq_sb[:, cs])
                nc.vector.tensor_mul(pkq[:], pkq[:], ssq_sb[:, cs])
                nc.vector.tensor_add(Q2[0:HS, cs], t1[:], pkq[:])
                nc.vector.tensor_copy(Q2[HS:128, cs], Q2[0:HS, cs])

            # ---- k/v chunk-pair machinery ----
            # chunk-pair p: k-proj lo -> psum rows 0:64 (t 512p..), hi -> rows
            # 64:128 (t 2048+512p..); one copy to kt; rope into K2f cols
            # [512p : 512p+512]; v-proj for t-tiles 4p..4p+3 and 16+4p..19+4p.
            def k_mm(p, half, ps, cis):
                rb = half * HS
                toff = half * 2048 + p * 512
                for ci in cis:
                    nc.tensor.matmul(
                        ps[rb:rb + HS, :],
                        lhsT=wk_sb[:, ci * HS:(ci + 1) * HS],
                        rhs=xt_sb[ci][:, toff:toff + 512],
                        start=(ci == 0), stop=(ci == NC4 - 1),
                    )

            def k_rope_pk(p, kt):
                pk = pkp.tile([128, 512], BF, tag="pk", name=f"pk{p}")
                for b0 in range(0, 128, 32):
                    b1 = b0 ^ 32  # partner block
                    nc.vector.tensor_copy(pk[b0:b0 + 32, :], kt[b1:b1 + 32, :])
                return pk

            def k_rope_mul(p, kt, pk):
                cs = slice(p * 512, (p + 1) * 512)
                t1 = pkp.tile([128, 512], BF, tag="t1k", name=f"t1k{p}")
                nc.vector.tensor_mul(t1[:], kt[:], cck_sb[:, cs])
                nc.vector.tensor_mul(pk[:], pk[:], ssk_sb[:, cs])
                nc.vector.tensor_add(K2f[:, cs], t1[:], pk[:])

            def v_mm_pair(p, half, q, ps):
                # 2 t-tiles into cols [q*128 : (q+1)*128] of a [128, 256] psum
                toff = half * 2048 + p * 512
                for t_ in (q * 2, q * 2 + 1):
                    for ci in range(NC4):
                        nc.tensor.matmul(
                            ps[:, t_ * HS:(t_ + 1) * HS],
                            lhsT=xt_sb[ci][:, toff + t_ * 128:toff + (t_ + 1) * 128],
                            rhs=wv_sb[:, ci * HS:(ci + 1) * HS],
                            start=(ci == 0), stop=(ci == NC4 - 1),
                        )

            def v_copy_quad(p, half, ps):
                vh = v_half[half]
                dst = vh[:, (p * 4) * 65:(p * 4 + 4) * 65]
                nc.vector.tensor_copy(
                    dst.rearrange("part (a hh) -> part a hh", a=4)[:, :, 0:HS],
                    ps[:].rearrange("part (a hh) -> part a hh", a=4),
                )

            # ---- attention steps ----
            # phase A = (0,P),(1,P) interleaved; phase B = all of sc2 then all
            # of sc3, so epilogue(2) overlaps sc3's steps and only epilogue(3)
            # is in the tail
            steps = [(sc, P) for P in range(NP) for sc in (0, 1)] + \
                    [(2, P) for P in range(NP)] + [(3, P) for P in range(NP)]
            psos = {}

            def scores(i):
                sc, P = steps[i]
                psw = pwp.tile([128, 1024], FP, tag="psw", name=f"psw{sc}_{P}")
                for half in range(2):
                    rb = half * HS
                    nc.tensor.matmul(
                        psw[:, half * 512:(half + 1) * 512],
                        lhsT=K2f[rb:rb + HS, P * 128:(P + 1) * 128],
                        rhs=Q2[rb:rb + HS, sc * 512:(sc + 1) * 512],
                        start=True, stop=True,
                    )
                et = esb.tile([128, 1024], BF, tag="et", name=f"et{sc}_{P}")
                return (psw, et)

            def expstep(i, pe):
                psw, et = pe
                nc.scalar.activation(et[:], psw[:], Exp, scale=SCALE)

            def att(i, pe):
                sc, P = steps[i]
                _, et = pe
                if sc not in psos:
                    psos[sc] = pop.tile([65, 512], FP, tag="pso", name=f"pso{sc}")
                for half in range(2):
                    tj = P + half * (NT // 2)
                    vh, vo = v_half[tj // (NT // 2)], (tj % (NT // 2)) * 65
                    nc.tensor.matmul(
                        psos[sc][:],
                        lhsT=vh[:, vo:vo + 65],
                        rhs=et[:, half * 512:(half + 1) * 512],
                        start=(P == 0 and half == 0), stop=(P == NP - 1 and half == 1),
                    )
                return (sc, P)

            def epilogue(sc):
                pso = psos.pop(sc)
                osb = osbp.tile([65, 512], BF, tag="osb", name=f"osb{sc}")
                nc.vector.tensor_copy(osb[:], pso[:])
                out_sb = osbp.tile([128, 4 * HS], FP, tag="outsb", name=f"outsb{sc}")
                for j in range(4):
                    pst = projp.tile([128, 65], BF, tag="proj", name=f"pst{sc}_{j}")
                    nc.tensor.transpose(
                        pst[:], osb[:, j * 128:(j + 1) * 128], identb[0:65, 0:65]
                    )
                    zr = osbp.tile([128, 1], FP, tag="zr", name=f"zr{sc}_{j}")
                    nc.vector.reciprocal(zr[:], pst[:, HS:HS + 1])
                    nc.vector.tensor_scalar_mul(
                        out_sb[:, j * HS:(j + 1) * HS], pst[:, 0:HS], zr[:]
                    )
                # out DMA halves on two idle rings (issue in parallel)
                for half, eng in ((0, nc.sync), (1, nc.gpsimd)):
                    eng.dma_start(
                        out[sc * 512 + half * 256:sc * 512 + (half + 1) * 256, :]
                        .rearrange("(j p) h -> p j h", p=128),
                        out_sb[:, half * 128:(half + 1) * 128]
                        .rearrange("p (j h) -> p j h", j=2),
                    )

            # extra work schedule: chunk-pair p is consumed starting at step
            # 8p (phase A); emit cp p's build during steps [8(p-1) .. 8p-1],
            # cp 0 in the prologue. q-rope chunks 2,3 early-mid phase A;
            # epilogues for sc0/sc1 as phase B begins (each must be emitted
            # before the att() that re-allocates its PSUM slot).
            _cp = {}

            # ---- prologue: q chunks 0,1 + chunk-pair 0 + v for cp 0 ----
            q_proj_chunk(0, nc.scalar.copy)
            q_proj_chunk(1, nc.scalar.copy)
            q_rope_chunk(0)
            kps0 = projp.tile([128, 512], FP, tag="proj", name="kps0")
            k_mm(0, 0, kps0, range(NC4))
            k_mm(0, 1, kps0, range(NC4))
            kt0 = ktp.tile([128, 512], BF, tag="kt", name="kt0")
            nc.scalar.copy(kt0[:], kps0[:])
            pk0 = k_rope_pk(0, kt0)
            k_rope_mul(0, kt0, pk0)
            q_rope_chunk(1)
            vps0l = projp.tile([128, 256], FP, tag="proj", name="vps0l")
            v_mm_pair(0, 0, 0, vps0l)
            v_mm_pair(0, 0, 1, vps0l)
            v_copy_quad(0, 0, vps0l)
            vps0h = projp.tile([128, 256], FP, tag="proj", name="vps0h")
            v_mm_pair(0, 1, 0, vps0h)
            v_mm_pair(0, 1, 1, vps0h)
            v_copy_quad(0, 1, vps0h)

            # ---- pipelined steps: scores one ahead of exp ----
            NSTEPS = len(steps)
            pe_cur = scores(0)
            pend_att = None
            for i in range(NSTEPS):
                pe_next = scores(i + 1) if i + 1 < NSTEPS else None
                expstep(i, pe_cur)
                # chunk-pair kt alloc bookkeeping happens inside extra; track
                # the kt tile handle for the rope 2 steps later
                if i < 24:
                    p, ph = i // 8 + 1, i % 8
                    if ph == 0:
                        _cp["kps"] = projp.tile([128, 512], FP, tag="proj",
                                                name=f"kps{p}")
                        k_mm(p, 0, _cp["kps"], (0, 1))
                    elif ph == 1:
                        k_mm(p, 0, _cp["kps"], (2, 3))
                    elif ph == 2:
                        k_mm(p, 1, _cp["kps"], (0, 1))
                    elif ph == 3:
                        k_mm(p, 1, _cp["kps"], (2, 3))
                        kt = ktp.tile([128, 512], BF, tag="kt", name=f"kt{p}")
                        nc.vector.tensor_copy(kt[:], _cp["kps"][:])
                        _cp["kt"] = kt
                    elif ph == 4:
                        _cp["pk"] = k_rope_pk(p, _cp["kt"])
                        _cp["vpsl"] = projp.tile([128, 256], FP, tag="proj",
                                                 name=f"vps{p}l")
                        v_mm_pair(p, 0, 0, _cp["vpsl"])
                    elif ph == 5:
                        k_rope_mul(p, _cp["kt"], _cp["pk"])
                        v_mm_pair(p, 0, 1, _cp["vpsl"])
                    elif ph == 6:
                        v_copy_quad(p, 0, _cp["vpsl"])
                        _cp["vpsh"] = projp.tile([128, 256], FP, tag="proj",
                                                 name=f"vps{p}h")
                        v_mm_pair(p, 1, 0, _cp["vpsh"])
                    elif ph == 7:
                        v_mm_pair(p, 1, 1, _cp["vpsh"])
                        v_copy_quad(p, 1, _cp["vpsh"])
                elif i == 25:
                    q_proj_chunk(2, nc.vector.tensor_copy)
                    q_rope_chunk(2)
                elif i == 27:
                    q_proj_chunk(3, nc.vector.tensor_copy)
                    q_rope_chunk(3)
                elif i == 33:
                    epilogue(0)
                elif i == 34:
                    epilogue(1)
                elif i == 49:
                    epilogue(2)
                if pend_att is not None:
                    att(*pend_att)
                pend_att = (i, pe_cur)
                pe_cur = pe_next
            att(*pend_att)
            epilogue(3)

    if split_waits:
        _split_excess_waits(nc)
    _prog_cache[key] = nc
    return nc


def make_in_maps(x_image, x_text_emb, freqs_latex, freqs_img_x, freqs_img_y, Wk, Wq, Wv):
    """Host-side prep: transpose/cast activations, permute+transpose weights,
    build rope cos/sin tables (k tables folded to [128, TK/2])."""
    perm = np.concatenate([np.arange(0, HS, 2), np.arange(1, HS, 2)])

    wk_dev = np.ascontiguousarray(np.asarray(Wk)[perm].T).astype(BF16)
    wq_dev = np.ascontiguousarray(np.asarray(Wq)[perm].T).astype(BF16)
    wv_dev = np.ascontiguousarray(np.asarray(Wv).T).astype(BF16)

    fx = np.asarray(freqs_img_x, dtype=np.float32)
    fy = np.asarray(freqs_img_y, dtype=np.float32)
    fl = np.asarray(freqs_latex, dtype=np.float32)
    ck_half = np.concatenate([fx[:, :, 0].T, fy[:, :, 0].T], axis=0)  # [32, TK]
    sk_half = np.concatenate([fx[:, :, 1].T, fy[:, :, 1].T], axis=0)
    cc64 = np.concatenate([ck_half, ck_half], 0)    # [64, TK]
    ss64 = np.concatenate([-sk_half, sk_half], 0)   # [64, TK]
    # fold: rows 0:64 = t 0..2047, rows 64:128 = t 2048..4095
    cck = np.ascontiguousarray(
        np.concatenate([cc64[:, :TK // 2], cc64[:, TK // 2:]], 0)).astype(BF16)
    ssk = np.ascontiguousarray(
        np.concatenate([ss64[:, :TK // 2], ss64[:, TK // 2:]], 0)).astype(BF16)
    cq_half = fl[:, :, 0].T  # [32, TQ]
    sq_half = fl[:, :, 1].T
    ccq = np.ascontiguousarray(np.concatenate([cq_half, cq_half], 0)).astype(BF16)
    ssq = np.ascontiguousarray(np.concatenate([-sq_half, sq_half], 0)).astype(BF16)

    xi = np.asarray(x_image, dtype=np.float32)
    xte = np.asarray(x_text_emb, dtype=np.float32)
    in_maps = []
    for b in range(N_CORES):
        in_maps.append(
            {
                "xt": np.ascontiguousarray(xi[b].T).astype(BF16),
                "xtt": np.ascontiguousarray(xte[b].T).astype(BF16),
                "wk": wk_dev, "wq": wq_dev, "wv": wv_dev,
                "cck": cck, "ssk": ssk, "ccq": ccq, "ssq": ssq,
            }
        )
    return in_maps


def kernel(x_image, x_text_emb, x_latex_mask, freqs_latex, freqs_img_x, freqs_img_y,
           Wk, Wq, Wv):
    del x_latex_mask  # unused in the reference
    from concourse.bass_utils import run_bass_kernel_spmd

    nc = build_program()
    in_maps = make_in_maps(
        x_image, x_text_emb, freqs_latex, freqs_img_x, freqs_img_y, Wk, Wq, Wv
    )
    res = run_bass_kernel_spmd(nc, in_maps, list(range(N_CORES)))
    return np.stack([res.results[b]["out"] for b in range(N_CORES)], axis=0)
